# revision 16
# baseline (speedup 1.0000x reference)
"""MultiHeadAttention (B=4, S=2048, D=1024, H=16, causal + key mask) on 8 trn2 cores.

Sharding: Megatron-style tensor parallel over heads. Each core owns 2 heads:
column slices of Wq/Wk/Wv (D x 128), the matching row slice of Wp (128 x D).
Each core computes a partial output y_c = attn_c @ Wp_c; host sums the 8
partials and adds bp.

Per-core kernel (all matmuls float32r: full PE rate at N=512, ~2e-4 rel err):
  - x^T [D, B*S] streamed in chunks; projections produce Q^T/K^T
    [128 = 2 heads x 64, B, S] directly (W slice as lhsT, x^T as rhs).
  - V via PE transpose into [s, hd] layout + a ones column so the PV matmul
    also accumulates the softmax denominator (row 64 of the PV psum).
  - Scores computed transposed: S^T[k, q] = K^T_slice.T @ Q^T_slice (K=64).
    Both heads' score blocks land in one [128,1024] 2-bank PSUM tile ->
    ONE additive causal mask (DVE) + ONE exp (ScalarE, key-mask as
    per-partition bias) per k-block. No max-subtraction (logits are O(1)).
  - PV accumulates attnT[hd, q]; psum copied to SBUF fast (frees the bank),
    reciprocal (DVE) -> partition-broadcast (DMA) -> normalize into a
    dedicated attnT buffer (reuses the x-stream pool's SBUF space).
  - Output projection is emitted two groups behind so the normalize chain
    never head-of-line blocks the in-order PE queue.
"""

import numpy as np

P = 128
B, S, D, H = 4, 2048, 1024, 16
HD = D // H  # 64
NCORES = 8
HPC = H // NCORES  # 2 heads per core
BS = B * S  # 8192
NB = S // P  # 16 k-blocks per batch
NG = S // 512  # 4 q-groups per batch

_CACHE = {}


def _build_nc():
    import concourse.mybir as mybir
    from concourse import bacc
    from concourse.tile import TileContext
    from concourse.masks import make_identity
    from contextlib import ExitStack

    f32 = mybir.dt.float32
    f32r = mybir.dt.float32r
    AF = mybir.ActivationFunctionType

    nc = bacc.Bacc("TRN2", target_bir_lowering=False, debug=False,
                   num_devices=NCORES)

    xT_d = nc.dram_tensor("xT", [D, BS], f32r, kind="ExternalInput").ap()
    wq_d = nc.dram_tensor("wq", [D, P], f32r, kind="ExternalInput").ap()
    wk_d = nc.dram_tensor("wk", [D, P], f32r, kind="ExternalInput").ap()
    wv_d = nc.dram_tensor("wv", [D, P], f32r, kind="ExternalInput").ap()
    bq_d = nc.dram_tensor("bq", [P, 1], f32, kind="ExternalInput").ap()
    bk_d = nc.dram_tensor("bk", [P, 1], f32, kind="ExternalInput").ap()
    bv_d = nc.dram_tensor("bv", [P, 1], f32, kind="ExternalInput").ap()
    wp_d = nc.dram_tensor("wp", [P, D], f32r, kind="ExternalInput").ap()
    mb_d = nc.dram_tensor("maskb", [P, B * NB], f32, kind="ExternalInput").ap()
    cm_d = nc.dram_tensor("cmask", [P, 4, 1024], f32,
                          kind="ExternalInput").ap()
    yp_d = nc.dram_tensor("yp", [BS, D], f32, kind="ExternalOutput").ap()

    xT_r = xT_d.rearrange("(o p) n -> p o n", p=P)  # [128, 8, 8192]
    KD = D // P  # 8 contraction chunks

    with TileContext(nc) as tc:
        with ExitStack() as ctx:
            consts = ctx.enter_context(tc.tile_pool(name="consts", bufs=1))
            big = ctx.enter_context(tc.tile_pool(name="big", bufs=1))
            ptpool = ctx.enter_context(tc.tile_pool(name="ptpool", bufs=3))
            npool = ctx.enter_context(tc.tile_pool(name="npool", bufs=2))
            ypool = ctx.enter_context(tc.tile_pool(name="ypool", bufs=3))
            psum = ctx.enter_context(
                tc.tile_pool(name="psum", bufs=2, space="PSUM"))
            sc2pool = ctx.enter_context(
                tc.tile_pool(name="sc2pool", bufs=2, space="PSUM"))
            pvpool = ctx.enter_context(
                tc.tile_pool(name="pvpool", bufs=2, space="PSUM"))

            # ---- constants ----
            wq_sb = consts.tile([P, KD, P], f32r, tag="wq")
            wk_sb = consts.tile([P, KD, P], f32r, tag="wk")
            wv_sb = consts.tile([P, KD, P], f32r, tag="wv")
            nc.sync.dma_start(wq_sb[:], wq_d.rearrange("(o p) m -> p o m", p=P))
            nc.sync.dma_start(wk_sb[:], wk_d.rearrange("(o p) m -> p o m", p=P))
            nc.sync.dma_start(wv_sb[:], wv_d.rearrange("(o p) m -> p o m", p=P))
            wp_sb = consts.tile([P, D], f32r, tag="wp")
            nc.sync.dma_start(wp_sb[:], wp_d)
            bq_sb = consts.tile([P, 1], f32, tag="bq")
            bk_sb = consts.tile([P, 1], f32, tag="bk")
            bv_sb = consts.tile([P, 1], f32, tag="bv")
            nc.sync.dma_start(bq_sb[:], bq_d)
            nc.sync.dma_start(bk_sb[:], bk_d)
            nc.sync.dma_start(bv_sb[:], bv_d)
            mb_sb = consts.tile([P, B * NB], f32, tag="mb")
            nc.sync.dma_start(mb_sb[:], mb_d)
            cm_sb = consts.tile([P, 4, 1024], f32, tag="cm")
            nc.sync.dma_start(cm_sb[:], cm_d)
            ident = consts.tile([P, P], f32, tag="ident")
            make_identity(nc, ident[:])

            # ---- persistent activations ----
            qt_sb = big.tile([P, B, S], f32r, tag="qt")  # Q^T
            kt_sb = big.tile([P, B, S], f32r, tag="kt")  # K^T
            # V in [s, hd] layout + ones col: [p=s%128, h, b, sblock, 65]
            v_sb = big.tile([P, HPC, B, NB, HD + 1], f32r, tag="v")
            nc.vector.memset(v_sb[:, :, :, :, HD].bitcast(f32), 1.0)

            # ---- phase 1: projections (x-stream pools scoped here) ----
            with tc.tile_pool(name="xpool", bufs=2) as xpool, \
                 tc.tile_pool(name="vtpool", bufs=2) as vtpool:
                for c in range(BS // 512):  # 16 chunks of 512 rows, b-major
                    b, sc = divmod(c, NG)
                    xt = xpool.tile([P, KD, 512], f32r, tag="xt")
                    nc.sync.dma_start(xt[:], xT_r[:, :, c * 512:(c + 1) * 512])
                    ssl = slice(sc * 512, (sc + 1) * 512)

                    for which in range(3):
                        w_sb = (wq_sb, wk_sb, wv_sb)[which]
                        ps = psum.tile([P, 512], f32, tag="ps")
                        for o in range(KD):
                            nc.tensor.matmul(
                                ps[:], lhsT=w_sb[:, o, :], rhs=xt[:, o, :],
                                start=(o == 0), stop=(o == KD - 1))
                        if which == 0:
                            nc.scalar.activation(qt_sb[:, b, ssl], ps[:],
                                                 AF.Identity, bias=bq_sb[:])
                        elif which == 1:
                            nc.scalar.activation(kt_sb[:, b, ssl], ps[:],
                                                 AF.Identity, bias=bk_sb[:])
                        else:
                            vt = vtpool.tile([P, 512], f32, tag="vt")
                            nc.scalar.activation(vt[:], ps[:], AF.Identity,
                                                 bias=bv_sb[:])
                            for t in range(4):
                                trp = psum.tile([P, 512], f32, tag="ps")
                                nc.tensor.transpose(
                                    trp[:, :P], vt[:, t * P:(t + 1) * P],
                                    ident[:])
                                sb_i = sc * 4 + t
                                nc.vector.tensor_copy(
                                    v_sb[:, 0, b, sb_i, 0:HD], trp[:, 0:HD])
                                nc.vector.tensor_copy(
                                    v_sb[:, 1, b, sb_i, 0:HD],
                                    trp[:, HD:2 * HD])

            # attnT buffer (reuses the closed x-stream pools' SBUF space)
            atpool = ctx.enter_context(tc.tile_pool(name="atpool", bufs=1))
            at_sb = atpool.tile([P, B, S], f32r, tag="at")

            # ---- phase 2: attention + output projection ----
            def outproj(b, g):
                for qc in range(4):
                    q0 = g * 512 + qc * P
                    r0 = b * S + q0
                    y_sb = ypool.tile([P, D], f32, tag="y",
                                      name=f"y_{b}_{g}_{qc}")
                    for half in range(2):
                        yp_ps = psum.tile([P, 512], f32, tag="ps",
                                          name=f"yps_{b}_{g}_{qc}_{half}")
                        nc.tensor.matmul(
                            yp_ps[:],
                            lhsT=at_sb[:, b, q0:q0 + P],
                            rhs=wp_sb[:, half * 512:(half + 1) * 512],
                            start=True, stop=True)
                        ysl = y_sb[:, half * 512:(half + 1) * 512]
                        if half == 0:
                            nc.vector.tensor_copy(ysl, yp_ps[:])
                        else:
                            nc.scalar.activation(ysl, yp_ps[:], AF.Copy)
                    nc.sync.dma_start(yp_d[r0:r0 + P, :], y_sb[:])

            pending = []
            for b in range(B):
                for g in range(NG):
                    gsl = slice(g * 512, (g + 1) * 512)
                    nkb = 4 * (g + 1)
                    pvs = [pvpool.tile([P, 512], f32, tag="pv",
                                       name=f"pv_{b}_{g}_{h}")
                           for h in range(HPC)]
                    for kb in range(nkb):
                        j = kb - 4 * g
                        col = b * NB + kb
                        sc2 = sc2pool.tile([P, 1024], f32, tag="sc2",
                                           name=f"sc2_{b}_{g}_{kb}")
                        for h in range(HPC):
                            hsl = slice(h * HD, (h + 1) * HD)
                            nc.tensor.matmul(
                                sc2[:, h * 512:(h + 1) * 512],
                                lhsT=kt_sb[hsl, b, kb * P:(kb + 1) * P],
                                rhs=qt_sb[hsl, b, gsl],
                                start=True, stop=True)
                        if j >= 0:  # diagonal block: additive causal mask
                            nc.vector.tensor_add(sc2[:], sc2[:],
                                                 cm_sb[:, j, :])
                        pt = ptpool.tile([P, 1024], f32r, tag="pt")
                        nc.scalar.activation(pt[:], sc2[:], AF.Exp,
                                             bias=mb_sb[:, col:col + 1])
                        for h in range(HPC):
                            nc.tensor.matmul(
                                pvs[h][0:HD + 1, :],
                                lhsT=v_sb[:, h, b, kb, :],
                                rhs=pt[:, h * 512:(h + 1) * 512],
                                start=(kb == 0), stop=(kb == nkb - 1))
                    if len(pending) >= 2:
                        outproj(*pending.pop(0))
                    pending.append((b, g))
                    for h in range(HPC):
                        # free the pv psum slot fast: copy [65,512] to SBUF
                        pvs_sb = npool.tile([P, 512], f32, tag="pvs")
                        nc.scalar.activation(pvs_sb[0:HD + 1, :],
                                             pvs[h][0:HD + 1, :], AF.Copy)
                        # 1/sum(exp) (row 64), broadcast to 64 partitions
                        rec = npool.tile([P, 512], f32, tag="rec")
                        nc.vector.reciprocal(
                            rec[HD:HD + 1, :], pvs_sb[HD:HD + 1, :])
                        sx = npool.tile([HD, 512], f32, tag="sx")
                        nc.sync.dma_start(
                            sx[:],
                            rec[HD:HD + 1, None, :]
                            .to_broadcast((1, HD, 512)))
                        if h == 0:
                            nc.vector.tensor_mul(
                                at_sb[0:HD, b, gsl], pvs_sb[0:HD, :], sx[:])
                        else:
                            tmp = npool.tile([HD, 512], f32r, tag="tmp")
                            nc.vector.tensor_mul(
                                tmp[:], pvs_sb[0:HD, :], sx[:])
                            nc.sync.dma_start(at_sb[HD:2 * HD, b, gsl],
                                              tmp[:])

            for pg in pending:
                outproj(*pg)

    nc.compile()
    return nc


def _get_nc():
    if "nc" not in _CACHE:
        _CACHE["nc"] = _build_nc()
    return _CACHE["nc"]


def make_in_maps(x, attention_mask, Wq, bq, Wk, bk, Wv, bv, Wp, bp):
    """Host-side sharding: build the 8 per-core device input maps."""
    x = np.asarray(x, dtype=np.float32)
    scale = np.float32(1.0 / np.sqrt(HD))
    xT = np.ascontiguousarray(x.reshape(BS, D).T)  # [D, BS]
    mb = (np.asarray(attention_mask).astype(np.float32) - 1.0) * np.float32(1e9)
    mb = np.ascontiguousarray(
        mb.reshape(B, NB, P).transpose(2, 0, 1).reshape(P, B * NB))
    # causal diag masks (additive): 0 where 128*j + p <= f, else -1e9;
    # duplicated for the two head halves of the [128,1024] scores tile.
    pp = np.arange(P)[:, None]
    ff = np.arange(512)[None, :]
    cm = np.stack(
        [np.where(P * j + pp <= ff, 0.0, -1e9).astype(np.float32)
         for j in range(4)], axis=1)  # [128, 4, 512]
    cm = np.ascontiguousarray(np.concatenate([cm, cm], axis=-1))

    Wq = np.asarray(Wq, np.float32) * scale
    bq = np.asarray(bq, np.float32) * scale
    Wk = np.asarray(Wk, np.float32)
    bk = np.asarray(bk, np.float32)
    Wv = np.asarray(Wv, np.float32)
    bv = np.asarray(bv, np.float32)
    Wp = np.asarray(Wp, np.float32)

    in_maps = []
    for c in range(NCORES):
        cs = slice(c * P, (c + 1) * P)
        in_maps.append({
            "xT": xT,
            "wq": np.ascontiguousarray(Wq[:, cs]),
            "wk": np.ascontiguousarray(Wk[:, cs]),
            "wv": np.ascontiguousarray(Wv[:, cs]),
            "bq": np.ascontiguousarray(bq[cs].reshape(P, 1)),
            "bk": np.ascontiguousarray(bk[cs].reshape(P, 1)),
            "bv": np.ascontiguousarray(bv[cs].reshape(P, 1)),
            "wp": np.ascontiguousarray(Wp[cs, :]),
            "maskb": mb,
            "cmask": cm,
        })
    return in_maps


def run(inputs, trace=False, tmpdir=None):
    """Compile (cached) + run on 8 cores. Returns (output, BassKernelResults)."""
    from concourse import bass_utils
    nc = _get_nc()
    in_maps = make_in_maps(**inputs)
    kwargs = {}
    if trace:
        kwargs = dict(trace=True, tmpdir=tmpdir)
    res = bass_utils.run_bass_kernel_spmd(
        nc, in_maps, core_ids=list(range(NCORES)), **kwargs)
    acc = np.zeros((BS, D), dtype=np.float64)
    for r in res.results:
        acc += r["yp"].astype(np.float64)
    out = (acc + np.asarray(inputs["bp"], np.float64)[None, :]).astype(
        np.float32)
    return out.reshape(B, S, D), res


def kernel(**inputs) -> np.ndarray:
    out, _ = run(inputs, trace=False)
    return out
